# revision 1
# baseline (speedup 1.0000x reference)
"""Multi-head causal self-attention on 8 trn2 NeuronCores.

Problem: x[4, 2048, 1024], 16 heads of 64 dims, causal softmax attention,
torch-Linear style projections (y = x @ W.T + b).

Sharding: core c = (batch b = c // 2, head-group g = c % 2). Each core
computes the attention output for batch b over heads [8g, 8g+8) and the
partial output projection for those heads' 512 value dims. The host sums
the two head-group partials per batch (the "all-reduce after W_O" of
tensor parallelism, done during unshard) and adds the rank-1 bias
corrections (bv @ Wo.T + bo), which commute with attention because
softmax rows sum to 1.

Device layouts (per core):
  xT   [1024, 2048]  x[b].T
  wqT  [1024, 512]   Wq[512g:512(g+1), :].T   (same for wkT, wvT)
  woT  [512, 1024]   Wo.T[512g:512(g+1), :]
  bq   [512]         bias shard (applied on device; bk same)
  y    [2048, 1024]  partial output (missing bv/bo rank-1 terms)

On-chip pipeline, interleaved over 512-wide column chunks:
  - QT/KT [dq, T] via weight-stationary fp32r matmuls (1 PE row/cycle;
    Q kept per-window only); V [T-slice, dv] via x-stationary matmuls,
    stored fp16 per head with a ones column so the P@V' matmul also
    produces the softmax denominators.
  - Scores transposed per head, s_T[k, q] = K Q.T (fp32r). Score pairs
    land in one 2-bank PSUM tile so a single ACT instruction
    exponentiates two k-chunks (ACT per-instruction overhead is the
    attention loop's scarcest resource), emitting fp16. In windows
    <= PAIR_WMAX the two partition-half heads' K=64 score matmuls are
    emitted adjacently per chunk: disjoint PE row groups let the array
    run them concurrently (row tiling, a hardware win the cost model
    does not credit).
  - The causal mask is a multiplicative 0/1 square applied after exp
    (off the scores->exp critical chain).
  - P@V' in fp16 with the exp tile stationary, sub-q-outer (one PSUM
    bank per accumulation group), lagging one head behind scores/exp so
    it never waits on ACT; projection/tail work fills the PE between
    heads. 1/denominator folds into the PSUM drain (vector engine).
  - The attention output is PE-transposed per sub-q chunk and fed to
    the fp32r W_O matmul. (fp32r transpose mode would save 3us but is
    unreachable: memset/affine_select lack f32r codegen and fp32-view
    identity producers fail the fp32r-producer check.)

Engine budget per core (cost model): PE ~211us busy / 80% duty
(projections 82, scores 62+, P@V 29, transposes 14, W_O 27), ACT
~161us (exp), DVE ~96us (psum drains, masks, rescales), GPSIMD ~53us
(fp32r rounding copies), DMA ~61us. Modeled span ~263us; remaining
idle is the ~12us DMA-bound startup ramp, the ~5us end barrier, and
the causal-staircase exp floor of the last window. Measured end-to-end
rel err 2.644e-4 (fp32r ~1.5e-4/matmul + fp16 P@V ~1.9e-4).
"""

from contextlib import ExitStack

import numpy as np

import concourse.bass as bass
import concourse.mybir as mybir
import concourse.tile as tile
from concourse import bacc
from concourse.masks import make_identity

F32 = mybir.dt.float32
F32R = mybir.dt.float32r
F16 = mybir.dt.float16
Exp = mybir.ActivationFunctionType.Exp
Identity = mybir.ActivationFunctionType.Identity
Copy = mybir.ActivationFunctionType.Copy

# fp32r ("rounded" fp32, ~13-bit mantissa) streams 1 PE row/cycle at free
# dim >= 256 vs 4 cycles/row for full fp32 — 4x matmul throughput. HW-
# measured rel err ~1.5e-4 per matmul. Operands must be produced by a
# rounding compute op (plain DMA into an fp32r tile fails birverifier).
USE_F32R = True
# windows <= PAIR_WMAX run head PAIRS with chunk-interleaved score matmuls
# (adjacent K=64 matmuls on disjoint PE row groups run concurrently in the
# array — a hardware win the cost model doesn't credit). Bounded by exp
# slots: the pair holds both heads' exp tiles live.
PAIR_WMAX = 1

D = 1024          # model dim
T = 2048          # sequence length
BATCH = 4
NH = 16           # total heads
DH = 64           # head dim
HLOC = 8          # heads per core
DSH = 512         # value dims per core (HLOC * DH)
N_CORES = 8

TC = T // 512     # 4 column tiles of 512
KC = T // 128     # 16 k chunks of 128
DC = D // 128     # 8 contraction chunks for the QKV projections


def _build(ablate=()):
    """ablate: subset of {"pv", "exp", "scores", "mask", "rescale", "tail"}
    — drop those instruction groups (timing studies only; output garbage)."""
    nc = bacc.Bacc("TRN2", target_bir_lowering=False, debug=False,
                   num_devices=N_CORES)
    xT = nc.dram_tensor("xT", [D, T], F32, kind="ExternalInput").ap()
    wqT = nc.dram_tensor("wqT", [D, DSH], F32, kind="ExternalInput").ap()
    wkT = nc.dram_tensor("wkT", [D, DSH], F32, kind="ExternalInput").ap()
    wvT = nc.dram_tensor("wvT", [D, DSH], F32, kind="ExternalInput").ap()
    woT = nc.dram_tensor("woT", [DSH, D], F32, kind="ExternalInput").ap()
    bq = nc.dram_tensor("bq", [DSH], F32, kind="ExternalInput").ap()
    bk = nc.dram_tensor("bk", [DSH], F32, kind="ExternalInput").ap()
    y = nc.dram_tensor("y", [T, D], F32, kind="ExternalOutput").ap()

    FMM = F32R if USE_F32R else F32  # dtype of fp32-class matmul operands

    with tile.TileContext(nc) as tc, ExitStack() as ctx:
        singles = ctx.enter_context(tc.tile_pool(name="singles", bufs=1))
        wpool = ctx.enter_context(tc.tile_pool(name="wpool", bufs=1))
        xtpool = ctx.enter_context(tc.tile_pool(name="xtpool", bufs=1))
        tmp_pool = ctx.enter_context(tc.tile_pool(name="tmp", bufs=5))
        qtpool = ctx.enter_context(tc.tile_pool(name="qt", bufs=2))
        attnp = ctx.enter_context(tc.tile_pool(name="attnp", bufs=2))
        attnTp = ctx.enter_context(tc.tile_pool(name="attnTp", bufs=2))
        # all of a window's exp tiles stay live (the PV loop runs sub-q-
        # outer, lagging one head); 13 double-width slots cover two heads
        # of window 2 plus lookahead
        exp_pool = ctx.enter_context(tc.tile_pool(name="exp", bufs=13))
        small = ctx.enter_context(tc.tile_pool(name="small", bufs=8))
        ybuf = ctx.enter_context(tc.tile_pool(name="ybuf", bufs=3))
        # PSUM: 4 (two double-bank score tiles: exp reads a k-chunk PAIR in
        # one scalar-engine instruction) + 2 (PV accumulators, sub-q-outer)
        # + 2 (fill: projection groups, attn transposes, W_O groups — all
        # emission-interleaved filler work)
        ps_s = ctx.enter_context(tc.tile_pool(name="ps_s", bufs=2, space="PSUM"))
        ps_pv = ctx.enter_context(tc.tile_pool(name="ps_pv", bufs=2, space="PSUM"))
        ps_fill = ctx.enter_context(tc.tile_pool(name="ps_fill", bufs=2, space="PSUM"))

        KT_t = singles.tile([128, 4, T], FMM)       # [dk%128, dk//128, t]
        Vp_t = singles.tile([128, KC, HLOC, DH + 1], F16)  # [t%128, t//128, h, dv+1]
        ident_t = singles.tile([128, 128], F32)
        mask_t = singles.tile([128, 128], F16)      # 0/1 causal square
        bq_t = singles.tile([128, 4], F32)
        bk_t = singles.tile([128, 4], F32)

        make_identity(nc, ident_t)
        nc.vector.memset(Vp_t[:, :, :, DH:DH + 1], 1.0)
        nc.gpsimd.memset(mask_t, 1.0)
        # s_T layout [k, q]: multiplicative 0/1 causal mask for the 128x128
        # diagonal square, applied to exp(s) AFTER the exp so the mask sits
        # off the scores->exp chain (exp(s)*0 == exp(s-1e6)). Keep 1.0
        # where (qq - kk) >= 0, else 0. (is_le is unimplemented in walrus
        # codegen, hence the negated is_ge form.)
        nc.gpsimd.affine_select(
            out=mask_t, in_=mask_t,
            compare_op=mybir.AluOpType.is_ge,
            fill=0.0,
            base=0,
            pattern=[[1, 128]],
            channel_multiplier=-1,
        )


        def load(dst, src):
            """DMA src into dst; fp32r dsts bounce through a temp tile + a
            rounding copy on gpsimd (birverifier requires fp32r matmul
            operands to be produced by a rounding compute op, not a DMA —
            in-place rounding over the DMA'd buffer is also rejected; gpsimd
            keeps the rounds off the DVE, whose psum drains sit on the
            attention critical path)."""
            if dst.dtype == F32R:
                stage = tmp_pool.tile([128, 512], F32, tag="stage", name="stage")
                nc.sync.dma_start(out=stage, in_=src)
                nc.gpsimd.tensor_copy(dst, stage)
            else:
                nc.sync.dma_start(out=dst, in_=src)

        # Wq/Wk live c-major ([dq-chunk, contraction-chunk, col]) so the
        # first Q/K projection groups (c=0) complete after a quarter of the
        # weight bytes land — the first scores and exp start ~10us earlier
        wq_t = wpool.tile([128, 4, DC, 128], FMM)
        wk_t = wpool.tile([128, 4, DC, 128], FMM)
        wv_t = wpool.tile([128, DC, DSH], FMM)
        wo_t = wpool.tile([128, 4, D], FMM)
        wqT_r = wqT.rearrange("(d p) (c j) -> p c d j", p=128, c=4)
        wkT_r = wkT.rearrange("(d p) (c j) -> p c d j", p=128, c=4)
        wvT_r = wvT.rearrange("(d p) j -> p d j", p=128)
        woT_r = woT.rearrange("(c p) j -> p c j", p=128)
        xT_r = xT.rearrange("(d p) t -> p d t", p=128)

        # emission order sets DMA/engine priority: x(0), then (Wq c, Wk c)
        # column-block pairs in the order the first window's heads need them
        xt0 = xtpool.tile([128, DC, 512], FMM, tag="xt", name="xt")
        for d in range(DC):
            load(xt0[:, d, :], xT_r[:, d, 0:512])
        # bias DMAs sit after x(0) on the serial DMA queue (they're tiny
        # strided transfers only needed at the first Q drain)
        nc.sync.dma_start(out=bq_t, in_=bq.rearrange("(c p) -> p c", p=128))
        nc.sync.dma_start(out=bk_t, in_=bk.rearrange("(c p) -> p c", p=128))
        for c in range(4):
            for w_t, w_r in ((wq_t, wqT_r), (wk_t, wkT_r)):
                for hf in range(2):
                    load(w_t[:, c, 4 * hf:4 * (hf + 1), :],
                         w_r[:, c, 4 * hf:4 * (hf + 1), :])
        for d in range(DC):
            load(wv_t[:, d, :], wvT_r[:, d, :])

        def proj_steps(w, box):
            """Closures emitting projection work for chunk w, finest-grain
            first: xt/qt alloc, Q groups (the attention window needs them
            first), K groups, then V groups — matching DMA data arrival so
            the PE's static instruction order never head-of-line blocks on
            a later weight load. box["qt"] is set by the first step."""
            steps = []

            def alloc(w=w):
                if w == 0:
                    xt = xt0
                else:
                    xt = xtpool.tile([128, DC, 512], FMM, tag="xt", name="xt")
                    for d in range(DC):
                        load(xt[:, d, :], xT_r[:, d, 512 * w:512 * (w + 1)])
                box["xt"] = xt
                box["qt"] = qtpool.tile([128, 4, 512], FMM, tag="qt", name="qt_w")
            steps.append(alloc)

            def qstep(c, w=w):
                xt, qt_w = box["xt"], box["qt"]
                psp = ps_fill.tile([128, 512], F32, tag="fill", name="psq")
                for d in range(DC):
                    nc.tensor.matmul(
                        psp,
                        lhsT=wq_t[:, c, d, :],
                        rhs=xt[:, d, :],
                        start=(d == 0), stop=(d == DC - 1),
                    )
                nc.vector.tensor_scalar_add(qt_w[:, c, :], psp, bq_t[:, c:c + 1])

            def kstep(c, w=w):
                xt = box["xt"]
                psk = ps_fill.tile([128, 512], F32, tag="fill", name="psk")
                for d in range(DC):
                    nc.tensor.matmul(
                        psk,
                        lhsT=wk_t[:, c, d, :],
                        rhs=xt[:, d, :],
                        start=(d == 0), stop=(d == DC - 1),
                    )
                nc.vector.tensor_scalar_add(
                    KT_t[:, c, 512 * w:512 * (w + 1)], psk, bk_t[:, c:c + 1])

            def vstep(s, w=w):
                xt = box["xt"]
                psv = ps_fill.tile([128, 512], F32, tag="fill", name="psv")
                for d in range(DC):
                    nc.tensor.matmul(
                        psv,
                        lhsT=xt[:, d, 128 * s:128 * (s + 1)],
                        rhs=wv_t[:, d, :],
                        start=(d == 0), stop=(d == DC - 1),
                    )
                nc.vector.tensor_copy(
                    Vp_t[:, 4 * w + s, :, 0:DH],
                    psv.rearrange("p (h v) -> p h v", h=HLOC),
                )

            for c in range(4):
                steps.append(lambda c=c: qstep(c))
                steps.append(lambda c=c: kstep(c))
            for s in range(4):
                steps.append(lambda s=s: vstep(s))
            return steps

        def emit_scores_exp(w, h, qt_w):
            kmax = 4 * (w + 1)
            ch, po = h // 2, (h % 2) * 64
            # scores for a PAIR of k-chunks land in one 2-bank PSUM tile so
            # a single scalar-engine instruction exponentiates both (ACT
            # per-instruction overhead is the attention loop's scarcest
            # resource). All of the window's exp tiles stay live so the PV
            # loop can run sub-q-outer, one head behind.
            ex_buf = []
            for jp in range(kmax // 2):
                pssb = ps_s.tile([128, 2, 512], F32, tag="pss", name="pss")
                exb = exp_pool.tile([128, 2, 512], F16, tag="ex", name="ex")
                rel0 = 2 * jp - 4 * w
                # both matmuls write from the PAIR's first live column (the
                # second diag chunk's extra 128 columns are garbage that exp
                # covers but PV never reads — writing them keeps the paired
                # exp's input region fully initialized)
                q0 = max(rel0, 0) * 128
                for sub in range(2):
                    j = 2 * jp + sub
                    if "scores" not in ablate:
                        nc.tensor.matmul(
                            pssb[:, sub, q0:],
                            lhsT=KT_t[po:po + 64, ch, 128 * j:128 * (j + 1)],
                            rhs=qt_w[po:po + 64, ch, q0:],
                            start=True, stop=True,
                        )
                # pairs are both-full or both-diagonal (diag chunks are the
                # last 4 and 4w is even). For a diag pair the exp covers
                # [128*rel0:512] of both chunks; chunk rel0+1's columns
                # [128*rel0:128*(rel0+1)] are garbage, but PV of sub-q i
                # only reads chunks with rel <= i, so they're never used.
                e0 = max(rel0, 0) * 128
                if "exp" not in ablate:
                    nc.scalar.activation(out=exb[:, :, e0:],
                                         in_=pssb[:, :, e0:],
                                         func=Exp, scale=0.125)
                if "mask" not in ablate:
                    for sub in range(2):
                        rel = 2 * jp + sub - 4 * w
                        if rel >= 0:
                            q0 = rel * 128
                            # zero exp(s) above the diagonal; only PV of
                            # sub-q i == rel reads this square
                            nc.vector.tensor_mul(
                                exb[:, sub, q0:q0 + 128],
                                exb[:, sub, q0:q0 + 128], mask_t)
                ex_buf.append((exb, 0))
                ex_buf.append((exb, 1))
            return ex_buf

        def emit_scores_exp_hpair(w, hp, qt_w):
            """Scores + exp for the head pair (2hp, 2hp+1), k-chunks of the
            two heads interleaved so adjacent K=64 score matmuls target
            disjoint PE row groups (partition halves 0-63 / 64-127) and run
            concurrently in the array. Needs 2x the exp slots of a single
            head while both heads' tiles accumulate."""
            kmax = 4 * (w + 1)
            ch = hp
            exA, exB = [], []
            for jp in range(kmax // 2):
                pA = ps_s.tile([128, 2, 512], F32, tag="pss", name="pss")
                pB = ps_s.tile([128, 2, 512], F32, tag="pss", name="pss")
                eA = exp_pool.tile([128, 2, 512], F16, tag="ex", name="ex")
                eB = exp_pool.tile([128, 2, 512], F16, tag="ex", name="ex")
                rel0 = 2 * jp - 4 * w
                q0 = max(rel0, 0) * 128
                for sub in range(2):
                    j = 2 * jp + sub
                    if "scores" not in ablate:
                        nc.tensor.matmul(
                            pA[:, sub, q0:],
                            lhsT=KT_t[0:64, ch, 128 * j:128 * (j + 1)],
                            rhs=qt_w[0:64, ch, q0:],
                            start=True, stop=True,
                        )
                        nc.tensor.matmul(
                            pB[:, sub, q0:],
                            lhsT=KT_t[64:128, ch, 128 * j:128 * (j + 1)],
                            rhs=qt_w[64:128, ch, q0:],
                            start=True, stop=True,
                        )
                for pss, exb in ((pA, eA), (pB, eB)):
                    if "exp" not in ablate:
                        nc.scalar.activation(out=exb[:, :, q0:],
                                             in_=pss[:, :, q0:],
                                             func=Exp, scale=0.125)
                    if "mask" not in ablate:
                        for sub in range(2):
                            rel = 2 * jp + sub - 4 * w
                            if rel >= 0:
                                qq = rel * 128
                                nc.vector.tensor_mul(
                                    exb[:, sub, qq:qq + 128],
                                    exb[:, sub, qq:qq + 128], mask_t)
                exA += [(eA, 0), (eA, 1)]
                exB += [(eB, 0), (eB, 1)]
            return exA, exB

        def emit_pv(w, h, ex_buf, attn_t):
            # P@V', one 128-query sub-chunk at a time: each accumulation
            # group owns one PSUM bank (bank-granular zero regions) and
            # only 2 are in flight
            for i in range(4):
                pso = ps_pv.tile([128, DH + 1], F32, tag="pso", name="pso")
                if "pv" not in ablate:
                    jlast = 4 * w + i
                    for j in range(jlast + 1):
                        exb, sub = ex_buf[j]
                        nc.tensor.matmul(
                            pso,
                            lhsT=exb[:, sub, 128 * i:128 * (i + 1)],
                            rhs=Vp_t[:, j, h, :],
                            start=(j == 0), stop=(j == jlast),
                        )
                if "rescale" not in ablate:
                    rec = small.tile([128, 1], F32, tag="rec", name="rec")
                    nc.vector.reciprocal(rec, pso[:, DH:DH + 1])
                    # attn = pv_psum * (1/denom), broadcast along dv
                    nc.vector.tensor_mul(
                        attn_t[:, i, DH * h:DH * (h + 1)],
                        pso[:, 0:DH],
                        rec.broadcast_to([128, DH]),
                    )

        def emit_tail(w, attn_t, last=False):
            """Transpose + W_O + store for window w, pipelined per 128-query
            sub-chunk. The final window's psum drains go to the scalar
            engine (idle by then) instead of DVE."""
            if "tail" in ablate:
                return
            drain = nc.scalar.copy if last else nc.vector.tensor_copy
            for i in range(4):
                atT = attnTp.tile([128, 4, 128], FMM, tag="attnT", name="attnT")
                pst = ps_fill.tile([128, 512], F32, tag="fill", name="pst")
                for c in range(4):
                    nc.tensor.transpose(
                        pst[:, 128 * c:128 * (c + 1)],
                        attn_t[:, i, 128 * c:128 * (c + 1)], ident_t)
                drain(atT, pst.rearrange("p (c q) -> p c q", c=4))
                for jc in range(2):
                    py = ps_fill.tile([128, 512], F32, tag="fill", name="py")
                    for c in range(4):
                        nc.tensor.matmul(
                            py,
                            lhsT=atT[:, c, :],
                            rhs=wo_t[:, c, 512 * jc:512 * (jc + 1)],
                            start=(c == 0), stop=(c == 3),
                        )
                    ysb = ybuf.tile([128, 512], F32, tag="ysb", name="ysb")
                    drain(ysb, py)
                    nc.sync.dma_start(
                        out=y[512 * w + 128 * i:512 * w + 128 * (i + 1),
                              512 * jc:512 * (jc + 1)],
                        in_=ysb,
                    )

        # Driver. Within windows 0-2, P@V lags one head behind scores/exp
        # (so PV never waits on the scalar engine) and projection/tail work
        # fills the PE between heads. Window 3 (exp-saturated, no
        # projection left) runs heads sequentially.
        box0 = {}
        steps0 = proj_steps(0, box0)
        for s in steps0[:9]:   # alloc, Q x4, K x4; V groups interleave below
            s()
        qt_cur = box0["qt"]
        pend = None            # (w, h, ex_buf, attn_t) awaiting PV
        attn_prev = None
        box = {}
        carry = []             # leftover V/Wo fill steps from the previous window
        for w in range(TC):
            # must-steps (alloc/Q/K) produce the NEXT window's scores inputs
            # and have to finish inside this window; V/Wo steps are only
            # needed by the next window's P@V and may spill past the next
            # scores (keeping ACT fed across the window transition)
            must, spill = [], []
            if w == 0:
                spill += steps0[9:]          # V(0) — consumed before PV(h0)
                for c in range(4):
                    for jc in range(2):
                        spill.append(lambda c=c, jc=jc: load(
                            wo_t[:, c, 512 * jc:512 * (jc + 1)],
                            woT_r[:, c, 512 * jc:512 * (jc + 1)]))
            if w + 1 < TC:
                nxt = proj_steps(w + 1, box)
                must += nxt[:9]
                spill += nxt[9:]
            if w == 0:
                # V(0)/Wo first: PV(0, h0) (emitted at h1) needs V(0)
                it = iter(spill[:12] + must + spill[12:])
                spill_late = spill[12:]
            else:
                it = iter(must + spill)
                spill_late = spill
            attn_t = attnp.tile([128, 4, DSH], F32, tag="attn", name="attn_t")
            lag = w < TC - 1
            if w <= PAIR_WMAX:
                # head-pair windows: adjacent K=64 score matmuls hit
                # disjoint PE row groups (array-level concurrency); the
                # pair's first head's P@V runs immediately, the second
                # head's lags to the next pair
                for hp in range(HLOC // 2):
                    exA, exB = emit_scores_exp_hpair(w, hp, qt_cur)
                    if hp == 0:
                        for s in carry:      # previous window's V leftovers
                            s()
                        carry = []
                    n_fill = 4 if (w == 0 and hp == 0) else 1
                    for _ in range(n_fill):  # V(0) fully precedes PV(h0)
                        s = next(it, None)
                        if s is not None:
                            s()
                    if pend is not None:
                        pw, ph, pex, pat = pend
                        emit_pv(pw, ph, pex, pat)
                        if ph == HLOC - 1:   # window pw fully rescaled
                            emit_tail(pw, pat)
                        pend = None
                    emit_pv(w, 2 * hp, exA, attn_t)
                    pend = (w, 2 * hp + 1, exB, attn_t)
                continue_heads = ()
            else:
                continue_heads = range(HLOC)
            for h in continue_heads:
                ex = emit_scores_exp(w, h, qt_cur)
                if h == 0:
                    for s in carry:          # previous window's V leftovers
                        s()
                    carry = []
                n_fill = 4 if (w == 0 and h == 0) else 1
                for _ in range(n_fill):      # V(0) fully precedes PV(h0)
                    s = next(it, None)
                    if s is not None:
                        s()
                if pend is not None:
                    pw, ph, pex, pat = pend
                    emit_pv(pw, ph, pex, pat)
                    if ph == HLOC - 1:       # window pw fully rescaled
                        emit_tail(pw, pat)
                    pend = None
                if lag:
                    pend = (w, h, ex, attn_t)
                else:
                    emit_pv(w, h, ex, attn_t)
            # force-finish unconsumed must-steps now (the next window's
            # scores need them); only trailing V steps may spill over
            rest = list(it)
            must_left = max(0, len(rest) - len(spill_late))
            for s in rest[:must_left]:
                s()
            carry = rest[must_left:]
            attn_prev = attn_t
            if w + 1 < TC:
                qt_cur = box["qt"]
                box = {}
        for s in carry:
            s()
        emit_tail(TC - 1, attn_prev, last=True)
    nc.compile()
    return nc


def shard_inputs(x, Wq, bq, Wk, bk, Wv, bv, Wo, bo):
    """Returns the 8 per-core input maps."""
    in_maps = []
    for c in range(N_CORES):
        b, g = c // 2, c % 2
        sl = slice(DSH * g, DSH * (g + 1))
        in_maps.append({
            "xT": np.ascontiguousarray(x[b].T),
            "wqT": np.ascontiguousarray(Wq[sl, :].T),
            "wkT": np.ascontiguousarray(Wk[sl, :].T),
            "wvT": np.ascontiguousarray(Wv[sl, :].T),
            "woT": np.ascontiguousarray(Wo.T[sl, :]),
            "bq": np.ascontiguousarray(bq[sl]),
            "bk": np.ascontiguousarray(bk[sl]),
        })
    return in_maps


def combine_outputs(results, bv, Wo, bo):
    """Sum head-group partials per batch + rank-1 bias corrections."""
    corr = (bv @ Wo.T + bo).astype(np.float32)  # [D]; exact because softmax
    y = np.empty((BATCH, T, D), dtype=np.float32)  # rows sum to 1
    for b in range(BATCH):
        y[b] = results[2 * b]["y"] + results[2 * b + 1]["y"] + corr
    return y


def run_sharded(inputs, trace=False):
    """Build, compile, run on cores 0-7. Returns (y_full, BassKernelResults)."""
    from concourse import bass_utils

    inputs = {k: np.asarray(v, dtype=np.float32) for k, v in inputs.items()}
    nc = _build()
    in_maps = shard_inputs(
        inputs["x"], inputs["Wq"], inputs["bq"], inputs["Wk"], inputs["bk"],
        inputs["Wv"], inputs["bv"], inputs["Wo"], inputs["bo"])
    res = bass_utils.run_bass_kernel_spmd(
        nc, in_maps, list(range(N_CORES)), trace=trace)
    y = combine_outputs(res.results, inputs["bv"], inputs["Wo"], inputs["bo"])
    return y, res


def kernel(**inputs):
    y, _ = run_sharded(inputs, trace=False)
    return y


if __name__ == "__main__":
    rng = np.random.default_rng(0)
    demo = {
        "x": rng.standard_normal((BATCH, T, D), dtype=np.float32),
        "Wq": rng.standard_normal((D, D), dtype=np.float32) * 0.02,
        "bq": np.zeros(D, np.float32),
        "Wk": rng.standard_normal((D, D), dtype=np.float32) * 0.02,
        "bk": np.zeros(D, np.float32),
        "Wv": rng.standard_normal((D, D), dtype=np.float32) * 0.02,
        "bv": np.zeros(D, np.float32),
        "Wo": rng.standard_normal((D, D), dtype=np.float32) * 0.02,
        "bo": np.zeros(D, np.float32),
    }
    out = kernel(**demo)
    print(out.shape, out.dtype)



# revision 23
# speedup vs baseline: 1.3804x; 1.3804x over previous
"""Multi-head causal self-attention on 8 trn2 NeuronCores.

Problem: x[4, 2048, 1024], 16 heads of 64 dims, causal softmax attention,
torch-Linear style projections (y = x @ W.T + b).

Sharding: core c = (batch b = c // 2, head-group g = c % 2). Each core
computes the attention output for batch b over heads [8g, 8g+8) and the
partial output projection for those heads' 512 value dims. The host sums
the two head-group partials per batch (the "all-reduce after W_O" of
tensor parallelism, done during unshard) and adds the rank-1 bias
corrections (bv @ Wo.T + bo), which commute with attention because
softmax rows sum to 1.

Numerics: the Q/K projections and the score matmuls run in fp8e4m3 with
perf_mode=DoubleRow (two 128-deep k-tiles per instruction at 0.5
cycles/row): score noise enters the softmax exponent (~1%) and averages
out in P@V, contributing ~2e-3 end-to-end. The V path, P@V, and the
output projection stay fp16 (their error hits the output linearly).
All quantization happens host-side, so weights/activations DMA at 1-2
bytes/elem straight into matmul operands (no on-chip staging).

Device layouts (per core):
  x8   [1024, 2048]  x[b].T in fp8 (Q/K projections)
  x16  [1024, 2048]  x[b].T in fp16 (V projection)
  wq8/wk8 [128, 4, 4, 2, 128] fp8, output-column-permuted (see below)
  wv16 [1024, 512]   Wv[512g:512(g+1), :].T fp16
  wo16 [512, 1024]   Wo.T[512g:512(g+1), :] fp16
  bqp/bkp [512]      bias shards, column-permuted like wq8/wk8
  y    [2048, 1024]  partial output (missing bv/bo rank-1 terms)

Column permutation: PSUM c-chunk p=32*i+l holds dq = 64*H + 32*S + l with
H = 4*(c//2)+i, S = c%2. One [128,512] drain per (c,w) then lands head
H's dh-half S on partitions 32i..32i+32 of a [128, 2, 512] fp8 tile whose
middle dim is the dh-half — exactly the DoubleRow two-k-tile layout the
score matmuls need (contraction 2x32=64 at tile_position row 32i).

On-chip pipeline, interleaved over 512-wide column chunks:
  - Q/K projections: 4 DoubleRow fp8 matmuls per (c,w); V: 8 fp16
    matmuls per (s,w), stored fp16 per head with a ones column so P@V
    also produces the softmax denominators.
  - Scores per head: DoubleRow fp8, k-chunk pairs landing in one 2-bank
    PSUM tile so a single ACT instruction exponentiates both (ACT is
    the kernel's bottleneck engine: ~166us of exp).
  - The causal mask is a multiplicative 0/1 square applied after exp
    (off the scores->exp critical chain, on DVE).
  - P@V' in fp16 with the exp tile stationary, sub-q-outer; PV lags
    behind scores/exp (depth 2 in window 0, 1 in windows 1-2, 0 in the
    last) so it never waits on ACT. 1/denominator folds into the PSUM
    drain (vector engine).
  - Projection/V/tail work is queued as fill steps consumed one per
    score pair, keeping the PE's static instruction order from
    head-of-line blocking ACT behind a long fill burst.
  - Window 3's PV+tail interleave per 128-query sub-chunk so only the
    last sub-chunk's chain trails the final exp.
"""

from collections import deque
from contextlib import ExitStack

import numpy as np

import concourse.bass as bass
import concourse.mybir as mybir
import concourse.tile as tile
from concourse import bacc
from concourse.masks import make_identity

F32 = mybir.dt.float32
F16 = mybir.dt.float16
F8 = mybir.dt.float8e4
F32R = mybir.dt.float32r
Exp = mybir.ActivationFunctionType.Exp
DR = mybir.MatmulPerfMode.DoubleRow

D = 1024          # model dim
T = 2048          # sequence length
BATCH = 4
NH = 16           # total heads
DH = 64           # head dim
HLOC = 8          # heads per core
DSH = 512         # value dims per core (HLOC * DH)
N_CORES = 8

TC = T // 512     # 4 column tiles of 512
KC = T // 128     # 16 k chunks of 128
DC = D // 128     # 8 contraction chunks for the QKV projections

# PV lag depth per window: how many heads' P@V trail their scores/exp.
PV_DEPTH = (5, 2, 2, 1)


def _col_perm():
    """dq' -> dq permutation for the Q/K projection output columns."""
    perm = np.empty(DSH, dtype=np.int64)
    for c in range(4):
        for i in range(4):
            for l in range(32):
                perm[128 * c + 32 * i + l] = 64 * (4 * (c // 2) + i) + 32 * (c % 2) + l
    return perm


def _build():
    nc = bacc.Bacc("TRN2", target_bir_lowering=False, debug=False,
                   num_devices=N_CORES)
    x8 = nc.dram_tensor("x8", [D, T], F8, kind="ExternalInput").ap()
    x16 = nc.dram_tensor("x16", [D, T], F16, kind="ExternalInput").ap()
    wq8 = nc.dram_tensor("wq8", [128, 4, 4, 2, 128], F8, kind="ExternalInput").ap()
    wk8 = nc.dram_tensor("wk8", [128, 4, 4, 2, 128], F8, kind="ExternalInput").ap()
    wv16 = nc.dram_tensor("wv16", [D, DSH], F16, kind="ExternalInput").ap()
    wo16 = nc.dram_tensor("wo16", [DSH, D], F16, kind="ExternalInput").ap()
    bqp = nc.dram_tensor("bqp", [DSH], F32, kind="ExternalInput").ap()
    bkp = nc.dram_tensor("bkp", [DSH], F32, kind="ExternalInput").ap()
    y = nc.dram_tensor("y", [T, D], F16, kind="ExternalOutput").ap()

    with tile.TileContext(nc) as tc, ExitStack() as ctx:
        singles = ctx.enter_context(tc.tile_pool(name="singles", bufs=1))
        wpool = ctx.enter_context(tc.tile_pool(name="wpool", bufs=1))
        x8pool = ctx.enter_context(tc.tile_pool(name="x8p", bufs=2))
        x16pool = ctx.enter_context(tc.tile_pool(name="x16p", bufs=2))
        qtpool = ctx.enter_context(tc.tile_pool(name="qt", bufs=2))
        attnp = ctx.enter_context(tc.tile_pool(name="attnp", bufs=3))
        attnTp = ctx.enter_context(tc.tile_pool(name="attnTp", bufs=2))
        exp_pool = ctx.enter_context(tc.tile_pool(name="exp", bufs=24))
        small = ctx.enter_context(tc.tile_pool(name="small", bufs=8))
        ybuf = ctx.enter_context(tc.tile_pool(name="ybuf", bufs=3))
        ps_s = ctx.enter_context(tc.tile_pool(name="ps_s", bufs=2, space="PSUM"))
        ps_pv = ctx.enter_context(tc.tile_pool(name="ps_pv", bufs=2, space="PSUM"))
        ps_fill = ctx.enter_context(tc.tile_pool(name="ps_fill", bufs=2, space="PSUM"))

        # [dk%128, dk//128, ktile, t]; ktile 1 is zeros (DoubleRow pads
        # the 64-deep score contraction to 2x64 at 0.5 cycles/row)
        KT_t = singles.tile([128, 4, 2, T], F8)
        Vp_t = singles.tile([128, KC, HLOC, DH + 1], F16)  # [t%128, t//128, h, dv+1]
        ident_t = singles.tile([128, 128], F32)
        mask_t = singles.tile([128, 128], F16)      # 0/1 causal square
        bq_t = singles.tile([128, 4], F32)
        bk_t = singles.tile([128, 4], F32)

        make_identity(nc, ident_t)
        nc.gpsimd.memset(KT_t[:, :, 1, 0:512], 0.0)
        nc.vector.memset(Vp_t[:, :, :, DH:DH + 1], 1.0)
        nc.gpsimd.memset(mask_t, 1.0)
        # s_T layout [k, q]: multiplicative 0/1 causal mask for the 128x128
        # diagonal square, applied to exp(s) AFTER the exp (exp(s)*0 ==
        # exp(s-1e6)). Keep 1.0 where (qq - kk) >= 0, else 0.
        nc.gpsimd.affine_select(
            out=mask_t, in_=mask_t,
            compare_op=mybir.AluOpType.is_ge,
            fill=0.0,
            base=0,
            pattern=[[1, 128]],
            channel_multiplier=-1,
        )

        wq8_t = wpool.tile([128, 4, 4, 2, 128], F8)
        wk8_t = wpool.tile([128, 4, 4, 2, 128], F8)
        wv16_t = wpool.tile([128, DC, DSH], F16)
        wo16_t = wpool.tile([128, 4, D], F16)
        wv16_r = wv16.rearrange("(d p) j -> p d j", p=128)
        wo16_r = wo16.rearrange("(c p) j -> p c j", p=128)
        x8_r = x8.rearrange("(d p) t -> p d t", p=128)
        x16_r = x16.rearrange("(d p) t -> p d t", p=128)

        # DMA emission order sets queue priority: x8(0), all four Q/K
        # weight column-blocks, biases, then wv16 and x16(0) in 128-token
        # chunks (each V step only reads its own 128 columns, so the
        # first V group unblocks after 1/4 of the x16 bytes land).
        x8t0 = x8pool.tile([128, DC, 512], F8, tag="x8", name="x8t")
        nc.sync.dma_start(out=x8t0[:, 0:4], in_=x8_r[:, 0:4, 0:512])
        nc.sync.dma_start(out=wq8_t[:, 0], in_=wq8[:, 0])
        nc.sync.dma_start(out=wk8_t[:, 0], in_=wk8[:, 0])
        nc.sync.dma_start(out=x8t0[:, 4:8], in_=x8_r[:, 4:8, 0:512])
        nc.sync.dma_start(out=bq_t, in_=bqp.rearrange("(c p) -> p c", p=128))
        nc.sync.dma_start(out=bk_t, in_=bkp.rearrange("(c p) -> p c", p=128))
        nc.sync.dma_start(out=wq8_t[:, 1:4], in_=wq8[:, 1:4])
        nc.sync.dma_start(out=wk8_t[:, 1:4], in_=wk8[:, 1:4])
        nc.sync.dma_start(out=wv16_t, in_=wv16_r)
        x16t0 = x16pool.tile([128, DC, 512], F16, tag="x16", name="x16t")
        for s2 in range(2):
            nc.sync.dma_start(out=x16t0[:, :, 256 * s2:256 * (s2 + 1)],
                              in_=x16_r[:, :, 256 * s2:256 * (s2 + 1)])

        from collections import defaultdict
        by_key = defaultdict(list)
        fills = deque()   # step dicts consumed by budget-paced feed()

        def step(weight, key, fn):
            st = {"wt": weight, "fn": fn, "done": False}
            by_key[key].append(st)
            return st

        def run_step(st):
            if not st["done"]:
                st["done"] = True
                st["fn"]()

        def force(key):
            for st in by_key.get(key, ()):
                run_step(st)

        def proj_steps(w, box):
            """Weighted fill steps for window w: x/qt alloc + Q/K groups
            (DoubleRow fp8, deadline-keyed per c-chunk) and V halves
            (fp16, 256 dv wide so no step exceeds ~900ns of PE time)."""

            def alloc(w=w):
                if w == 0:
                    box["x8"], box["x16"] = x8t0, x16t0
                else:
                    x8t = x8pool.tile([128, DC, 512], F8, tag="x8", name="x8t")
                    nc.sync.dma_start(out=x8t, in_=x8_r[:, :, 512 * w:512 * (w + 1)])
                    x16t = x16pool.tile([128, DC, 512], F16, tag="x16", name="x16t")
                    for s2 in range(2):
                        nc.sync.dma_start(
                            out=x16t[:, :, 256 * s2:256 * (s2 + 1)],
                            in_=x16_r[:, :, 512 * w + 256 * s2:512 * w + 256 * (s2 + 1)])
                    box["x8"], box["x16"] = x8t, x16t
                qt_w = qtpool.tile([128, 4, 2, 512], F8, tag="qt", name="qt_w")
                nc.gpsimd.memset(qt_w[:, :, 1, :], 0.0)
                box["qt"] = qt_w

            def qkstep(c, wt, dst_f):
                x8t = box["x8"]
                psp = ps_fill.tile([128, 512], F32, tag="fill", name="psqk")
                for jp in range(4):
                    nc.tensor.matmul(
                        psp,
                        lhsT=wt[:, c, jp],
                        rhs=x8t[:, 2 * jp:2 * jp + 2, :],
                        start=(jp == 0), stop=(jp == 3),
                        perf_mode=DR,
                    )
                dst_f(c, psp)

            def qdrain(c, psp):
                nc.vector.tensor_scalar_add(box["qt"][:, c, 0, :], psp,
                                            bq_t[:, c:c + 1])

            def kdrain(c, psp, w=w):
                nc.vector.tensor_scalar_add(
                    KT_t[:, c, 0, 512 * w:512 * (w + 1)], psp, bk_t[:, c:c + 1])

            def vhalf(s, hf, w=w):
                x16t = box["x16"]
                psv = ps_fill.tile([128, 256], F32, tag="fill", name="psv")
                for d in range(DC):
                    nc.tensor.matmul(
                        psv,
                        lhsT=x16t[:, d, 128 * s:128 * (s + 1)],
                        rhs=wv16_t[:, d, 256 * hf:256 * (hf + 1)],
                        start=(d == 0), stop=(d == DC - 1),
                    )
                nc.vector.tensor_copy(
                    Vp_t[:, 4 * w + s, 4 * hf:4 * (hf + 1), 0:DH],
                    psv.rearrange("p (h v) -> p h v", h=4),
                )

            qk = {}
            qk[0] = [step(550, ("qk", w, 0), lambda: qkstep(0, wq8_t, qdrain)),
                     step(550, ("qk", w, 0), lambda: qkstep(0, wk8_t, kdrain))]
            for c in range(1, 4):
                qk[c] = [step(550, ("qk", w, c),
                              lambda c=c: qkstep(c, wq8_t, qdrain)),
                         step(550, ("qk", w, c),
                              lambda c=c: qkstep(c, wk8_t, kdrain))]
            vs = [step(900, ("v", w, s), lambda s=s, hf=hf: vhalf(s, hf))
                  for s in range(4) for hf in range(2)]
            # interleave: c-group deadlines are heads 2c, V(w) is needed by
            # the first P@V pop of window w. The alloc step (x DMA issue)
            # is returned separately -- the driver runs it immediately at
            # the previous window's start so the transfers land in time.
    

            return (step(0, ("qk", w, 0), alloc),
                    qk[0] + qk[1] + vs[0:2] + qk[2] + vs[2:4]
                    + qk[3] + vs[4:8])

        def emit_scores_exp(w, h, qt, feed):
            """DoubleRow fp8 scores + paired exp for head h of window w.
            Calls feed() after each pair (fill-step pacing). Returns the
            list of (exp_tile, sub) chunk handles."""
            kmax = 4 * (w + 1)
            ch, po = h // 2, (h % 2) * 64
            ex_buf = []
            for jp in range(kmax // 2):
                pssb = ps_s.tile([128, 2, 512], F32, tag="pss", name="pss")
                exb = exp_pool.tile([128, 2, 512], F16, tag="ex", name="ex")
                rel0 = 2 * jp - 4 * w
                # both matmuls write from the PAIR's first live column (the
                # second diag chunk's extra 128 columns are causally dead but
                # keep the paired exp's input region initialized)
                q0 = max(rel0, 0) * 128
                for sub in range(2):
                    j = 2 * jp + sub
                    nc.tensor.matmul(
                        pssb[:, sub, q0:],
                        lhsT=KT_t[po:po + 64, ch, :, 128 * j:128 * (j + 1)],
                        rhs=qt[po:po + 64, ch, :, q0:],
                        start=True, stop=True,
                        perf_mode=DR,
                    )
                nc.scalar.activation(out=exb[:, :, q0:],
                                     in_=pssb[:, :, q0:],
                                     func=Exp, scale=0.125)
                for sub in range(2):
                    rel = 2 * jp + sub - 4 * w
                    if rel >= 0:
                        qq = rel * 128
                        # zero exp(s) above the diagonal; only PV of
                        # sub-q i == rel reads this square
                        nc.vector.tensor_mul(
                            exb[:, sub, qq:qq + 128],
                            exb[:, sub, qq:qq + 128], mask_t)
                ex_buf.append((exb, 0))
                ex_buf.append((exb, 1))
                # pair's ACT time minus its PE time funds the fill budget
                feed((2 * (512 - q0)) * 0.833 + 185.0
                     - (2 * (512 - q0)) * 0.417)
            return ex_buf

        def emit_pv_sub(w, h, ex_buf, attn_t, i):
            """P@V' + rescale for one 128-query sub-chunk."""
            pso = ps_pv.tile([128, DH + 1], F32, tag="pso", name="pso")
            jlast = 4 * w + i
            for j in range(jlast + 1):
                exb, sub = ex_buf[j]
                nc.tensor.matmul(
                    pso,
                    lhsT=exb[:, sub, 128 * i:128 * (i + 1)],
                    rhs=Vp_t[:, j, h, :],
                    start=(j == 0), stop=(j == jlast),
                )
            rec = small.tile([128, 1], F32, tag="rec", name="rec")
            nc.vector.reciprocal(rec, pso[:, DH:DH + 1])
            nc.vector.tensor_mul(
                attn_t[:, i, DH * h:DH * (h + 1)],
                pso[:, 0:DH],
                rec.broadcast_to([128, DH]),
            )

        def emit_pv(w, h, ex_buf, attn_t):
            for i in range(4):
                emit_pv_sub(w, h, ex_buf, attn_t, i)

        def tail_sub(w, attn_t, i, last=False):
            """Transpose + W_O + store for one 128-query sub-chunk."""
            drain = nc.scalar.copy if last else nc.vector.tensor_copy
            atT = attnTp.tile([128, 4, 128], F16, tag="attnT", name="attnT")
            pst = ps_fill.tile([128, 512], F32, tag="fill", name="pst")
            for c in range(4):
                nc.tensor.transpose(
                    pst[:, 128 * c:128 * (c + 1)],
                    attn_t[:, i, 128 * c:128 * (c + 1)], ident_t)
            drain(atT, pst.rearrange("p (c q) -> p c q", c=4))
            ysb = ybuf.tile([128, 2, 512], F16, tag="ysb", name="ysb")
            for jc in range(2):
                py = ps_fill.tile([128, 512], F32, tag="fill", name="py")
                for c in range(4):
                    nc.tensor.matmul(
                        py,
                        lhsT=atT[:, c, :],
                        rhs=wo16_t[:, c, 512 * jc:512 * (jc + 1)],
                        start=(c == 0), stop=(c == 3),
                    )
                drain(ysb[:, jc, :], py)
            eng = nc.sync if last else nc.gpsimd
            eng.dma_start(
                out=y[512 * w + 128 * i:512 * w + 128 * (i + 1), :],
                in_=ysb,
            )

        # ---- driver ----
        pend = deque()    # (w, h, ex_buf, attn_t) awaiting P@V
        prio = deque()    # lazy P@V sub-chunk steps, drained before fills
        budget = [0.0]    # fill-step PE budget (ns), fed per score pair

        def feed(gain=611.0):
            budget[0] = min(budget[0] + gain, 1500.0)
            for q in (prio, fills):
                while q:
                    st = q[0]
                    if st["done"]:
                        q.popleft()
                        continue
                    if st["wt"] <= budget[0]:
                        budget[0] -= st["wt"]
                        q.popleft()
                        run_step(st)
                    else:
                        return

        def force_pv_all(pw):
            for ph in range(HLOC):
                force(("pv", pw, ph))

        def pv_sub_forced(pw, ph, pex, pat, i):
            for s in range(i + 1):
                force(("v", pw, s))
            emit_pv_sub(pw, ph, pex, pat, i)

        def enqueue_pv(pw, ph, pex, pat):
            """Queue P@V for (pw, ph) as 4 small prio steps (one per
            128-query sub-chunk) so feed() interleaves them into the
            ACT-paced score-pair stream."""
            if pw >= 2:
                force(("tail", pw - 2))   # attn ring (bufs=3) safety
            for i in range(4):
                prio.append(step(
                    (4 * pw + i + 1) * 27 + 60, ("pv", pw, ph),
                    lambda i=i: pv_sub_forced(pw, ph, pex, pat, i)))
            if ph == HLOC - 1:     # window pw fully rescaled -> tail it
                force(("wo",))
                for i in range(4):
                    def tfn(pw=pw, pat=pat, i=i):
                        force_pv_all(pw)
                        tail_sub(pw, pat, i)
                    fills.append(step(900, ("tail", pw), tfn))

        box0 = {}
        a0, s0 = proj_steps(0, box0)
        run_step(a0)
        # deferred KT zero tail: after qt(0)'s memset in alloc(0), before
        # window 1's scores need tokens 512+
        nc.gpsimd.memset(KT_t[:, :, 1, 512:T], 0.0)
        force(("qk", 0, 0))        # Q/K c0: first scores input
        qt_cur = box0["qt"]
        fills.extend(s0)           # remaining w0 steps (done ones skipped)
        box = {}
        a1, nxt = proj_steps(1, box)
        run_step(a1)               # x8/x16(1) DMAs queue behind w0 loads
        fills.extend(nxt)
        wo_st = step(0, ("wo",),
                     lambda: nc.sync.dma_start(out=wo16_t, in_=wo16_r))
        run_step(wo_st)

        for w in range(TC):
            if 1 <= w < TC - 1:
                a2, s2 = proj_steps(w + 1, box)
                run_step(a2)       # issue x8/x16(w+1) DMAs now
                fills.extend(s2)
            if w >= 3:
                force(("tail", w - 3))   # attn ring slot reuse (bufs=3)
            attn_t = attnp.tile([128, 4, DSH], F32, tag="attn", name="attn_t")
            depth = PV_DEPTH[w]
            for h in range(HLOC):
                force(("qk", w, h // 2))
                last_head = (w == TC - 1 and h == HLOC - 1)
                if last_head:
                    # hand pending P@V to the prio queue so it overlaps
                    # this head's pairs
                    while pend:
                        pw, ph, pex, pat = pend.popleft()
                        enqueue_pv(pw, ph, pex, pat)
                ex = emit_scores_exp(w, h, qt_cur, feed)
                if last_head:
                    for st in list(prio):
                        run_step(st)
                    prio.clear()
                    for s in range(4):
                        force(("v", w, s))
                    force(("wo",))
                    # interleave PV+tail per sub-chunk; sub-q 2/3 both wait
                    # on the final exp pair, so emit their PVs together and
                    # pipeline the two tails
                    for i in range(2):
                        emit_pv_sub(w, h, ex, attn_t, i)
                        tail_sub(w, attn_t, i, last=True)
                    emit_pv_sub(w, h, ex, attn_t, 2)
                    emit_pv_sub(w, h, ex, attn_t, 3)
                    tail_sub(w, attn_t, 2, last=True)
                    tail_sub(w, attn_t, 3, last=True)
                    continue
                pend.append((w, h, ex, attn_t))
                npop = 0
                while len(pend) > depth and npop < 2:
                    npop += 1
                    pw, ph, pex, pat = pend.popleft()
                    enqueue_pv(pw, ph, pex, pat)
            if w + 1 < TC:
                force(("qk", w + 1, 0))  # next window's first scores input
                qt_cur = box["qt"]
                box = {}
        for st in list(prio) + list(fills):
            run_step(st)
    nc.compile()
    return nc


def shard_inputs(x, Wq, bq, Wk, bk, Wv, bv, Wo, bo):
    """Returns the 8 per-core input maps (host-side quantization)."""
    import ml_dtypes
    f8 = ml_dtypes.float8_e4m3fn
    in_maps = []
    for c in range(N_CORES):
        b, g = c // 2, c % 2
        sl = slice(DSH * g, DSH * (g + 1))
        xT = np.ascontiguousarray(x[b].T)

        def pack_qk(W):
            a = W[sl].T.reshape(4, 2, 128, 4, 128)  # [jp, i, p, c, q]
            return np.ascontiguousarray(a.transpose(2, 3, 0, 1, 4)).astype(f8)

        in_maps.append({
            "x8": xT.astype(f8),
            "x16": xT.astype(np.float16),
            "wq8": pack_qk(Wq),
            "wk8": pack_qk(Wk),
            "wv16": np.ascontiguousarray(Wv[sl].T).astype(np.float16),
            "wo16": np.ascontiguousarray(Wo.T[sl]).astype(np.float16),
            "bqp": np.ascontiguousarray(bq[sl]).astype(np.float32),
            "bkp": np.ascontiguousarray(bk[sl]).astype(np.float32),
        })
    return in_maps


def combine_outputs(results, bv, Wo, bo):
    """Sum head-group partials per batch + rank-1 bias corrections."""
    corr = (bv @ Wo.T + bo).astype(np.float32)  # [D]; exact because softmax
    y = np.empty((BATCH, T, D), dtype=np.float32)  # rows sum to 1
    for b in range(BATCH):
        y[b] = (results[2 * b]["y"].astype(np.float32)
                + results[2 * b + 1]["y"].astype(np.float32) + corr)
    return y


def run_sharded(inputs, trace=False):
    """Build, compile, run on cores 0-7. Returns (y_full, BassKernelResults)."""
    from concourse import bass_utils

    inputs = {k: np.asarray(v, dtype=np.float32) for k, v in inputs.items()}
    nc = _build()
    in_maps = shard_inputs(
        inputs["x"], inputs["Wq"], inputs["bq"], inputs["Wk"], inputs["bk"],
        inputs["Wv"], inputs["bv"], inputs["Wo"], inputs["bo"])
    res = bass_utils.run_bass_kernel_spmd(
        nc, in_maps, list(range(N_CORES)), trace=trace)
    y = combine_outputs(res.results, inputs["bv"], inputs["Wo"], inputs["bo"])
    return y, res


def kernel(**inputs):
    y, _ = run_sharded(inputs, trace=False)
    return y


if __name__ == "__main__":
    rng = np.random.default_rng(0)
    demo = {
        "x": rng.standard_normal((BATCH, T, D), dtype=np.float32),
        "Wq": rng.standard_normal((D, D), dtype=np.float32) * 0.02,
        "bq": np.zeros(D, np.float32),
        "Wk": rng.standard_normal((D, D), dtype=np.float32) * 0.02,
        "bk": np.zeros(D, np.float32),
        "Wv": rng.standard_normal((D, D), dtype=np.float32) * 0.02,
        "bv": np.zeros(D, np.float32),
        "Wo": rng.standard_normal((D, D), dtype=np.float32) * 0.02,
        "bo": np.zeros(D, np.float32),
    }
    out = kernel(**demo)
    print(out.shape, out.dtype)
